# revision 2
# baseline (speedup 1.0000x reference)
"""Trainium2 Bass kernel for nn_DiracGraphConv (GNN edge-softmax message passing).

Block-matmul design (v2, no scatter):
  - Host normalizes z (zh), shards edges by destination: core k owns rows
    [k*12544, (k+1)*12544); blocks of 128 rows; superblocks of 7 blocks.
  - Tokens (edges) are grouped by (block b, col-group g=col&3) and padded to
    128-token units U; the per-(b,g) unit count is the max over cores so all 8
    cores share one SPMD program.
  - Per chunk (<=CH_U units, single g): one dma_gather of 512B rows
    [zh(64) | x(64) | 1 | pad] bf16 from the node table (window = cols==g mod 4
    via elem_step).
  - Per unit: PE-transpose the gathered zh -> zh_c^T, PE matmul with the
    block's host-transposed zh slab -> Mt[tok,row] (all-pairs logits) in PSUM;
    batched exp on ACT; one-hot row mask S built by iota/is_equal on DVE;
    W = exp(Mt)*S; PE matmul W^T @ [x|1] accumulates messages+denom for the
    block directly in PSUM. Pad tokens gather an all-zero table row so they
    contribute nothing.
  - Superblock epilogue: out = (msg/(denom+eps)) @ W^T + b via PE.
"""

import sys

sys.path.insert(0, "/opt/trn_rl_repo")

from dataclasses import dataclass

import numpy as np

from concourse import bacc, bass, mybir, tile
from concourse.library_config import mlp as MLP_LIB
from concourse.masks import make_identity

P = 128
F32 = mybir.dt.float32
BF16 = mybir.dt.bfloat16
I16 = mybir.dt.int16
EPS_DENOM = 1e-9


@dataclass(frozen=True)
class Cfg:
    n_cores: int = 8
    n_nodes: int = 100000
    d: int = 64
    npc: int = 12544  # nodes per core; cores 0-6 full, core 7 partial
    n_groups: int = 4
    sb_blocks: int = 7  # blocks per superblock (acc psum bank holds 7*65 f32)
    # max 128-token units per gather chunk: HW SWDGE handles <1024
    # descriptors per instruction, so 7*128=896 is the clean maximum
    ch_u: int = 7
    ring: int = 65536  # SWDGE descriptor ring bytes (4096 descs)
    group_cap: int = 25088  # local col index cap; null idx = group_cap-1

    @property
    def blocks(self) -> int:
        return self.npc // P

    @property
    def null_lidx(self) -> int:
        # null table row = 4*null_lidx + g >= n_nodes must hold
        assert 4 * (self.group_cap - 1) >= self.n_nodes
        return self.group_cap - 1

    @property
    def table_rows(self) -> int:
        return 4 * self.group_cap


FULL = Cfg()


def make_structure(U: np.ndarray, cfg: Cfg):
    """U[b, g] = number of 128-token units for (block, group).
    Returns (TU, superblocks) where superblocks is a list (one per sb) of
    chunk lists; each chunk is (g, u0, units) with units a list of
    (block_local_slot, start_flag, stop_flag) per unit."""
    nb_blocks = cfg.blocks
    n_sb = nb_blocks // cfg.sb_blocks
    # global unit index in stream order: sb -> g -> b -> u
    TU = int(U.sum())
    # start/stop flags are per-superblock: the first matmul into an acc bank
    # zeroes the whole 2KB region, the last one closes the group
    superblocks = []
    u0 = 0
    for sb in range(n_sb):
        blocks = range(sb * cfg.sb_blocks, (sb + 1) * cfg.sb_blocks)
        sb_total = int(U[sb * cfg.sb_blocks : (sb + 1) * cfg.sb_blocks].sum())
        assert sb_total > 0, "superblock with no units"
        chunks = []
        seen = 0
        for g in range(cfg.n_groups):
            units = []  # (slot, is_first, is_last) in stream order
            for b in blocks:
                for _ in range(int(U[b, g])):
                    units.append(
                        (b - sb * cfg.sb_blocks, seen == 0, seen == sb_total - 1)
                    )
                    seen += 1
            # split into chunks of <= ch_u
            i = 0
            while i < len(units):
                k = min(cfg.ch_u, len(units) - i)
                chunks.append((g, u0 + i, units[i : i + k]))
                i += k
            u0 += len(units)
        assert seen == sb_total
        superblocks.append(chunks)
    return TU, superblocks


def build_program(cfg: Cfg, U: np.ndarray, alpha: float):
    TU, superblocks = make_structure(U, cfg)
    D = cfg.d
    CH = cfg.ch_u

    nc = bacc.Bacc(
        "TRN2",
        target_bir_lowering=False,
        debug=False,
        num_swdge_queues=1,
        dynamic_dma_scratch_size=cfg.ring,
    )

    tbl = nc.dram_tensor(
        "tbl", [cfg.group_cap, 4, 256], BF16, kind="ExternalInput"
    ).ap()
    zhT = nc.dram_tensor("zhT", [D, cfg.npc], BF16, kind="ExternalInput").ap()
    cidx = nc.dram_tensor("cidx", [P, TU * 8], I16, kind="ExternalInput").ap()
    rowt = nc.dram_tensor("rowt", [P, TU], F32, kind="ExternalInput").ap()
    wb = nc.dram_tensor("wb", [D + 1, D], BF16, kind="ExternalInput").ap()
    out = nc.dram_tensor("out", [cfg.npc, D], F32, kind="ExternalOutput").ap()
    out_v = out.rearrange("(b p) d -> p b d", p=P)

    with tile.TileContext(nc) as tc:
        with (
            tc.tile_pool(name="const", bufs=1) as cpool,
            tc.tile_pool(name="gath", bufs=5) as gpool,
            tc.tile_pool(name="esw", bufs=6) as epool,
            tc.tile_pool(name="wp", bufs=4) as wpool,
            tc.tile_pool(name="zct", bufs=3) as zpool,
            tc.tile_pool(name="fin", bufs=2) as fpool,
            tc.tile_pool(name="mt", bufs=2, space="PSUM") as mpool,
            tc.tile_pool(name="tp", bufs=2, space="PSUM") as tpool,
            tc.tile_pool(name="acc", bufs=2, space="PSUM") as apool,
            tc.tile_pool(name="ep", bufs=1, space="PSUM") as ppool,
        ):
            nc.gpsimd.load_library(MLP_LIB)
            # ---- constants / preloads ----
            cb = cpool.tile([P, 1], F32, tag="cb")
            nc.vector.memset(cb[:], -abs(float(alpha)))
            ident = cpool.tile([P, P], BF16, tag="ident")
            make_identity(nc, ident[:])
            iota2 = cpool.tile([P, P], BF16, tag="iota2")
            nc.gpsimd.iota(
                iota2[:],
                pattern=[[1, P]],
                base=0,
                channel_multiplier=0,
                allow_small_or_imprecise_dtypes=True,
            )
            wbs = cpool.tile([D + 1, D], BF16, tag="wbs")
            nc.sync.dma_start(out=wbs[:], in_=wb[:, :])
            zhT_sb = cpool.tile([D, cfg.npc], BF16, tag="zhT")
            nc.sync.dma_start(out=zhT_sb[:], in_=zhT[:, :])
            cidx_sb = cpool.tile([P, TU * 8], I16, tag="cidx")
            nc.sync.dma_start(out=cidx_sb[:], in_=cidx[:, :])
            rowt_sb = cpool.tile([P, TU], F32, tag="rowt")
            nc.sync.dma_start(out=rowt_sb[:], in_=rowt[:, :])

            # flatten chunks across superblocks for gather prefetch
            all_chunks = []
            for sb, chunks in enumerate(superblocks):
                for ch in chunks:
                    all_chunks.append((sb, ch))

            def issue_gather(ci):
                _, (g_, u0_, units_) = all_chunks[ci]
                nb_ = len(units_)
                gj_ = gpool.tile([P, CH, 256], BF16, tag="gj")
                nc.gpsimd.dma_gather(
                    gj_[:, :nb_, :],
                    tbl[:, g_, :],
                    cidx_sb[:, u0_ * 8 : (u0_ + nb_) * 8],
                    nb_ * P,
                    nb_ * P,
                    256,
                    elem_step=1024,
                    queue_num=0,
                )
                return gj_

            def epilogue(sb, acc):
                accv = acc.rearrange("p (b c) -> p b c", c=D + 1)
                nsb = cfg.sb_blocks
                dpl = fpool.tile([P, nsb], F32, tag="dpl")
                nc.vector.tensor_scalar_add(
                    dpl[:, :], accv[:, :, D], EPS_DENOM
                )
                rr = fpool.tile([P, nsb], F32, tag="rr")
                nc.vector.reciprocal(out=rr[:, :], in_=dpl[:, :])
                m = fpool.tile([P, nsb, D + 1], BF16, tag="m")
                nc.vector.tensor_tensor(
                    out=m[:, :, 0:D],
                    in0=accv[:, :, 0:D],
                    in1=rr[:, :].to_broadcast([P, nsb, D]),
                    op=mybir.AluOpType.mult,
                )
                nc.vector.memset(m[:, :, D : D + 1], 1.0)
                o = fpool.tile([P, nsb, D], F32, tag="o")
                for j in range(nsb):
                    tp2 = ppool.tile([D + 1, P], BF16, tag="tp2", space="PSUM")
                    nc.tensor.transpose(
                        out=tp2[:], in_=m[:, j, :], identity=ident[:]
                    )
                    lhs = zpool.tile([D + 1, P], BF16, tag="lhs")
                    nc.vector.tensor_copy(out=lhs[:], in_=tp2[:])
                    y = ppool.tile([P, D], F32, tag="y", space="PSUM")
                    nc.tensor.matmul(
                        out=y[:], lhsT=lhs[:], rhs=wbs[:], start=True, stop=True
                    )
                    nc.scalar.copy(out=o[:, j, :], in_=y[:])
                nc.sync.dma_start(
                    out=out_v[:, sb * nsb : (sb + 1) * nsb, :], in_=o[:, :, :]
                )

            PREFETCH = 2
            gj_q = [
                issue_gather(i) for i in range(min(PREFETCH, len(all_chunks)))
            ]
            state = {"sb": -1, "acc": None}

            def consume(rec):
                # mask-multiply + message matmuls, one chunk behind the
                # produce stage so their exp-waits never head-of-line block
                # the next chunk's ready DVE/PE work
                sb, u0, units, gj, Es, Ss = rec
                if sb != state["sb"]:
                    if state["sb"] >= 0:
                        epilogue(state["sb"], state["acc"])
                    acc_t = apool.tile(
                        [P, cfg.sb_blocks * (D + 1)], F32, tag="acc"
                    )
                    state["acc"] = acc_t
                    state["sb"] = sb
                acc = state["acc"]
                nb = len(units)
                Ws = []
                for qi, i in enumerate(range(0, nb, 4)):
                    k = min(4, nb - i)
                    W = wpool.tile([P, 4, P], BF16, tag="W")
                    Ws.append(W)
                    nc.vector.tensor_tensor(
                        out=W[:, :k, :],
                        in0=Es[qi][:, :k, :],
                        in1=Ss[qi][:, :k, :],
                        op=mybir.AluOpType.mult,
                    )
                for u, (slot, first, last) in enumerate(units):
                    nc.tensor.matmul(
                        out=acc[:, slot * (D + 1) : (slot + 1) * (D + 1)],
                        lhsT=Ws[u // 4][:, u % 4, :],
                        rhs=gj[:, u, D : 2 * D + 1],
                        start=first,
                        stop=last,
                    )

            pending = None
            for ci, (sb, (g, u0, units)) in enumerate(all_chunks):
                gj = gj_q.pop(0)
                if ci + PREFETCH < len(all_chunks):
                    gj_q.append(issue_gather(ci + PREFETCH))
                nb = len(units)
                # transpose batch: 7 bf16 [64,128] transposes fill one
                # 2KB psum bank; one copy moves them to SBUF for lhsT
                zcts = []
                for i in range(0, nb, 8):
                    k = min(8, nb - i)
                    tp8 = tpool.tile([D, 8, P], BF16, tag="tp", space="PSUM")
                    for j in range(k):
                        nc.tensor.transpose(
                            out=tp8[:, j, :],
                            in_=gj[:, i + j, 0:D],
                            identity=ident[:],
                        )
                    zct8 = zpool.tile([D, 8, P], BF16, tag="zct")
                    nc.vector.tensor_copy(out=zct8[:, :k, :], in_=tp8[:, :k, :])
                    zcts.append(zct8)
                # S masks first: no deps, keeps DVE queue flowing
                Ss = []
                for i in range(0, nb, 4):
                    k = min(4, nb - i)
                    S = epool.tile([P, 4, P], BF16, tag="S")
                    Ss.append(S)
                    for j in range(k):
                        u = i + j
                        # 4x DVE mode (scalar_tensor_tensor gets none)
                        nc.vector.tensor_scalar(
                            out=S[:, j, :],
                            in0=iota2[:],
                            scalar1=rowt_sb[:, u0 + u : u0 + u + 1],
                            scalar2=None,
                            op0=mybir.AluOpType.is_equal,
                        )
                Es = []
                for i in range(0, nb, 4):
                    k = min(4, nb - i)
                    mtp = mpool.tile([P, 512], F32, tag="mtp", space="PSUM")
                    for j in range(k):
                        u = i + j
                        slot, _, _ = units[u]
                        b = sb * cfg.sb_blocks + slot
                        nc.tensor.matmul(
                            out=mtp[:, j * P : (j + 1) * P],
                            lhsT=zcts[u // 8][:, u % 8, :],
                            rhs=zhT_sb[:, b * P : (b + 1) * P],
                            start=True,
                            stop=True,
                        )
                    E = epool.tile([P, 4, P], BF16, tag="E")
                    Es.append(E)
                    nc.scalar.activation(
                        out=E[:, :k, :],
                        in_=mtp[:, : k * P],
                        func=mybir.ActivationFunctionType.Exp,
                        bias=cb[:],
                        scale=float(alpha),
                    )
                rec = (sb, u0, units, gj, Es, Ss)
                if pending is not None:
                    consume(pending)
                pending = rec
            consume(pending)
            epilogue(state["sb"], state["acc"])

    nc.compile()
    return nc, TU


def _wrap16(a: np.ndarray) -> np.ndarray:
    w = a.reshape(-1, 16).T
    return np.ascontiguousarray(np.tile(w, (8, 1)))


def _to_bf16(a: np.ndarray) -> np.ndarray:
    import ml_dtypes

    return np.asarray(a, np.float32).astype(ml_dtypes.bfloat16)


def shard_inputs(cfg: Cfg, x, z, edge_index):
    D = cfg.d
    row = np.asarray(edge_index[0], dtype=np.int64)
    col = np.asarray(edge_index[1], dtype=np.int64)

    # degree-balanced block assignment: snake-deal nodes (by in-degree desc)
    # across all core*block bins, then repair-pass so nearly every
    # (bin, group) load is <= 4*128 edges (keeps units/segment at 4)
    nbins = cfg.n_cores * cfg.blocks
    total = nbins * P
    g_edge = (col & 3).astype(np.int64)
    deg4 = np.zeros((cfg.n_nodes, 4), np.int64)
    np.add.at(deg4, (row, g_edge), 1)
    deg = deg4.sum(axis=1)
    order = np.argsort(-deg, kind="stable")
    pos = np.arange(cfg.n_nodes)
    rnd = pos // nbins
    idx = pos % nbins
    bin_snake = np.where(rnd % 2 == 0, idx, nbins - 1 - idx)
    bin_of = np.empty(cfg.n_nodes, np.int64)
    bin_of[order] = bin_snake

    L = np.zeros((nbins, 4), np.int64)
    for gg in range(4):
        L[:, gg] = np.bincount(
            bin_of, weights=deg4[:, gg].astype(np.float64), minlength=nbins
        ).astype(np.int64)
    # swap-repair toward per-(bin,g) caps: most bins capped at 4 units worth
    # of edges; a few designated "tall" bins (cap 5 units) concentrate the
    # overflow so nearly every segment ends at U=4.
    caps = np.full((nbins, 4), 4 * P, np.int64)
    if nbins >= 64:
        taken = np.zeros(nbins, bool)
        for gg in range(4):
            cnt = 0
            for bb in np.argsort(-L[:, gg]):
                if not taken[bb]:
                    taken[bb] = True
                    caps[bb, gg] = 5 * P
                    cnt += 1
                    if cnt == 24:
                        break
    bin_rows = [list(np.where(bin_of == bb)[0]) for bb in range(nbins)]
    for _ in range(40):
        over = np.where((L > caps).any(axis=1))[0]
        if over.size == 0:
            break
        swapped = 0
        for bb in over:
            guard = 0
            while (L[bb] > caps[bb]).any() and guard < 30:
                guard += 1
                gbad = int(np.argmax(L[bb] - caps[bb]))
                r1 = max(bin_rows[bb], key=lambda r: deg4[r, gbad])
                d1 = deg4[r1]
                done = False
                slack = caps[:, gbad] - L[:, gbad]
                for dst in np.argsort(-slack)[:100]:
                    dst = int(dst)
                    if dst == bb or slack[dst] <= 0:
                        continue
                    cand2 = sorted(
                        bin_rows[dst], key=lambda r: deg4[r, gbad]
                    )[:6]
                    for r2 in cand2:
                        d2 = deg4[r2]
                        newL_dst = L[dst] + d1 - d2
                        if (newL_dst <= caps[dst]).all() and d1[gbad] > d2[gbad]:
                            bin_rows[bb].remove(r1)
                            bin_rows[dst].remove(r2)
                            bin_rows[bb].append(r2)
                            bin_rows[dst].append(r1)
                            L[bb] += d2 - d1
                            L[dst] = newL_dst
                            swapped += 1
                            done = True
                            break
                    if done:
                        break
                if not done:
                    break
        if swapped == 0:
            break

    # pair bins into (core, block) slots by unit-pattern so the per-(b,g)
    # max over cores matches each bin's own ceil pattern
    ceil_pat = (L + P - 1) // P  # [nbins, 4]
    pat_key = (
        ceil_pat[:, 0] * 1000000
        + ceil_pat[:, 1] * 10000
        + ceil_pat[:, 2] * 100
        + ceil_pat[:, 3]
    )
    bin_order = np.argsort(pat_key, kind="stable")
    # slot s (0..97) of core k gets bin bin_order[s*8+k]
    new_pos = np.empty(cfg.n_nodes, np.int64)
    for s in range(cfg.blocks):
        for k in range(cfg.n_cores):
            bb = int(bin_order[s * cfg.n_cores + k])
            rs = bin_rows[bb]
            base = (k * cfg.blocks + s) * P
            new_pos[rs] = base + np.arange(len(rs))
    perm = new_pos
    inv_perm = np.full(total, -1, np.int64)
    inv_perm[new_pos] = np.arange(cfg.n_nodes)

    row = perm[row]
    core = row // cfg.npc
    local = row - core * cfg.npc
    blk = local >> 7
    rt = (local & 127).astype(np.float32)
    g = (col & 3).astype(np.int64)
    lidx = (col >> 2).astype(np.int64)

    nbg = cfg.blocks * cfg.n_groups
    key = (blk * cfg.n_groups + g).astype(np.int64)
    counts = np.zeros((cfg.n_cores, nbg), np.int64)
    for k in range(cfg.n_cores):
        sel = core == k
        counts[k] = np.bincount(key[sel], minlength=nbg)
    cmax = counts.max(axis=0).reshape(cfg.blocks, cfg.n_groups)
    U = (cmax + P - 1) // P
    # every superblock needs at least one unit so its psum acc region is
    # started (start=True zeroes the whole bank)
    n_sb = cfg.blocks // cfg.sb_blocks
    for sb in range(n_sb):
        sl = slice(sb * cfg.sb_blocks, (sb + 1) * cfg.sb_blocks)
        if U[sl].sum() == 0:
            U[sb * cfg.sb_blocks, 0] = 1
    TU = int(U.sum())

    # global unit offsets in stream order (sb -> g -> b)
    seg_u0 = np.zeros((cfg.blocks, cfg.n_groups), np.int64)
    u0 = 0
    n_sb = cfg.blocks // cfg.sb_blocks
    for sb in range(n_sb):
        for gg in range(cfg.n_groups):
            for b in range(sb * cfg.sb_blocks, (sb + 1) * cfg.sb_blocks):
                seg_u0[b, gg] = u0
                u0 += int(U[b, gg])
    assert u0 == TU
    T = TU * P

    cidx_cores, rowt_cores = [], []
    for k in range(cfg.n_cores):
        sel = core == k
        kk = key[sel]
        order = np.argsort(kk, kind="stable")
        ks = kk[order]
        rank = np.arange(ks.size) - np.searchsorted(ks, ks)
        b_s = ks // cfg.n_groups
        g_s = ks % cfg.n_groups
        tokpos = seg_u0[b_s, g_s] * P + rank
        ci = np.full(T, cfg.null_lidx, np.int16)
        rw = np.zeros(T, np.float32)
        ci[tokpos] = lidx[sel][order].astype(np.int16)
        rw[tokpos] = rt[sel][order]
        cidx_cores.append(_wrap16(ci))
        rowt_cores.append(np.ascontiguousarray(rw.reshape(-1, P).T))

    zf = np.asarray(z, np.float32)
    nrm = np.maximum(np.sqrt((zf * zf).sum(axis=1)), 1e-9)
    zh = zf / nrm[:, None]
    xf = np.asarray(x, np.float32)

    import ml_dtypes

    tbl = np.zeros((cfg.table_rows, 256), ml_dtypes.bfloat16)
    tbl[: cfg.n_nodes, 0:D] = _to_bf16(zh)
    tbl[: cfg.n_nodes, D : 2 * D] = _to_bf16(xf)
    tbl[: cfg.n_nodes, 2 * D] = np.ones(cfg.n_nodes, ml_dtypes.bfloat16)
    tbl = tbl.reshape(cfg.group_cap, 4, 256)

    # zh rows in permuted order (phantom tail rows stay zero)
    zh_pad = np.zeros((total, D), np.float32)
    real = inv_perm >= 0
    zh_pad[real] = zh[inv_perm[real]]

    in_maps = []
    for k in range(cfg.n_cores):
        zslab = zh_pad[k * cfg.npc : (k + 1) * cfg.npc]
        in_maps.append(
            {
                "tbl": tbl,
                "zhT": _to_bf16(np.ascontiguousarray(zslab.T)),
                "cidx": cidx_cores[k],
                "rowt": rowt_cores[k],
            }
        )
    return in_maps, U, TU, perm


def prep_wb(W, b):
    wb = np.concatenate(
        [np.asarray(W, np.float32).T, np.asarray(b, np.float32)[None, :]], axis=0
    )
    return _to_bf16(np.ascontiguousarray(wb))


def run(cfg: Cfg, x, edge_index, z, W, b, alpha, bias_edge, trace=False):
    from concourse.bass_utils import run_bass_kernel_spmd

    in_maps, U, TU, perm = shard_inputs(cfg, x, z, edge_index)
    wbv = prep_wb(W, b)
    for m in in_maps:
        m["wb"] = wbv
    nc, _ = build_program(cfg, U, float(np.asarray(alpha)))
    core_ids = list(range(cfg.n_cores))
    res = run_bass_kernel_spmd(nc, in_maps, core_ids, trace=trace)
    outs = [res.results[k]["out"] for k in core_ids]
    out = np.concatenate(outs, axis=0)[perm]
    return np.ascontiguousarray(out).astype(np.float32), res


def kernel(**inputs) -> np.ndarray:
    out, _ = run(
        FULL,
        inputs["x"],
        inputs["edge_index"],
        inputs["z"],
        inputs["W"],
        inputs["b"],
        inputs["alpha"],
        inputs["bias_edge"],
    )
    return out


# revision 8
# speedup vs baseline: 1.0122x; 1.0122x over previous
"""Trainium2 Bass kernel for nn_DiracGraphConv (GNN edge-softmax message passing).

Block-matmul design (v2, no scatter):
  - Host normalizes z (zh), shards edges by destination: core k owns rows
    [k*12544, (k+1)*12544); blocks of 128 rows; superblocks of 7 blocks.
  - Tokens (edges) are grouped by (block b, col-group g=col&3) and padded to
    128-token units U; the per-(b,g) unit count is the max over cores so all 8
    cores share one SPMD program.
  - Per chunk (<=CH_U units, single g): one dma_gather of 512B rows
    [zh(64) | x(64) | 1 | pad] bf16 from the node table (window = cols==g mod 4
    via elem_step).
  - Per unit: PE-transpose the gathered zh -> zh_c^T, PE matmul with the
    block's host-transposed zh slab -> Mt[tok,row] (all-pairs logits) in PSUM;
    batched exp on ACT; one-hot row mask S built by iota/is_equal on DVE;
    W = exp(Mt)*S; PE matmul W^T @ [x|1] accumulates messages+denom for the
    block directly in PSUM. Pad tokens gather an all-zero table row so they
    contribute nothing.
  - Superblock epilogue: out = (msg/(denom+eps)) @ W^T + b via PE.
"""

import sys

sys.path.insert(0, "/opt/trn_rl_repo")

from dataclasses import dataclass

import numpy as np

from concourse import bacc, bass, mybir, tile
from concourse.library_config import mlp as MLP_LIB
from concourse.masks import make_identity

P = 128
F32 = mybir.dt.float32
BF16 = mybir.dt.bfloat16
I16 = mybir.dt.int16
EPS_DENOM = 1e-9


@dataclass(frozen=True)
class Cfg:
    n_cores: int = 8
    n_nodes: int = 100000
    d: int = 64
    npc: int = 12544  # nodes per core; cores 0-6 full, core 7 partial
    n_groups: int = 4
    sb_blocks: int = 7  # blocks per superblock (acc psum bank holds 7*65 f32)
    # max 128-token units per gather chunk: HW SWDGE handles <1024
    # descriptors per instruction, so 7*128=896 is the clean maximum
    ch_u: int = 7
    ring: int = 65536  # SWDGE descriptor ring bytes (4096 descs)
    group_cap: int = 25088  # local col index cap; null idx = group_cap-1

    @property
    def blocks(self) -> int:
        return self.npc // P

    @property
    def null_lidx(self) -> int:
        # null table row = 4*null_lidx + g >= n_nodes must hold
        assert 4 * (self.group_cap - 1) >= self.n_nodes
        return self.group_cap - 1

    @property
    def table_rows(self) -> int:
        return 4 * self.group_cap


FULL = Cfg()


def make_structure(U: np.ndarray, cfg: Cfg):
    """U[b, g] = number of 128-token units for (block, group).
    Returns (TU, superblocks) where superblocks is a list (one per sb) of
    chunk lists; each chunk is (g, u0, units) with units a list of
    (block_local_slot, start_flag, stop_flag) per unit."""
    nb_blocks = cfg.blocks
    n_sb = nb_blocks // cfg.sb_blocks
    # global unit index in stream order: sb -> g -> b -> u
    TU = int(U.sum())
    # start/stop flags are per-superblock: the first matmul into an acc bank
    # zeroes the whole 2KB region, the last one closes the group
    superblocks = []
    u0 = 0
    for sb in range(n_sb):
        blocks = range(sb * cfg.sb_blocks, (sb + 1) * cfg.sb_blocks)
        sb_total = int(U[sb * cfg.sb_blocks : (sb + 1) * cfg.sb_blocks].sum())
        assert sb_total > 0, "superblock with no units"
        chunks = []
        seen = 0
        for g in range(cfg.n_groups):
            units = []  # (slot, is_first, is_last) in stream order
            for b in blocks:
                for _ in range(int(U[b, g])):
                    units.append(
                        (b - sb * cfg.sb_blocks, seen == 0, seen == sb_total - 1)
                    )
                    seen += 1
            # split into chunks of <= ch_u
            i = 0
            while i < len(units):
                k = min(cfg.ch_u, len(units) - i)
                chunks.append((g, u0 + i, units[i : i + k]))
                i += k
            u0 += len(units)
        assert seen == sb_total
        superblocks.append(chunks)
    return TU, superblocks


def build_program(cfg: Cfg, U: np.ndarray, alpha: float):
    TU, superblocks = make_structure(U, cfg)
    D = cfg.d
    CH = cfg.ch_u

    nc = bacc.Bacc(
        "TRN2",
        target_bir_lowering=False,
        debug=False,
        num_swdge_queues=1,
        dynamic_dma_scratch_size=cfg.ring,
    )

    tbl = nc.dram_tensor(
        "tbl", [cfg.group_cap, 4, 128], BF16, kind="ExternalInput"
    ).ap()
    zhT = nc.dram_tensor("zhT", [D, cfg.npc], BF16, kind="ExternalInput").ap()
    cidx = nc.dram_tensor("cidx", [P, TU * 8], I16, kind="ExternalInput").ap()
    rowt = nc.dram_tensor("rowt", [P, TU], F32, kind="ExternalInput").ap()
    wb = nc.dram_tensor("wb", [D + 1, D], BF16, kind="ExternalInput").ap()
    out = nc.dram_tensor("out", [cfg.npc, D], F32, kind="ExternalOutput").ap()
    out_v = out.rearrange("(b p) d -> p b d", p=P)

    with tile.TileContext(nc) as tc:
        with (
            tc.tile_pool(name="const", bufs=1) as cpool,
            tc.tile_pool(name="gath", bufs=5) as gpool,
            tc.tile_pool(name="esw", bufs=6) as epool,
            tc.tile_pool(name="wp", bufs=4) as wpool,
            tc.tile_pool(name="zct", bufs=3) as zpool,
            tc.tile_pool(name="fin", bufs=2) as fpool,
            tc.tile_pool(name="mt", bufs=2, space="PSUM") as mpool,
            tc.tile_pool(name="tp", bufs=2, space="PSUM") as tpool,
            tc.tile_pool(name="acc", bufs=2, space="PSUM") as apool,
            tc.tile_pool(name="ep", bufs=1, space="PSUM") as ppool,
        ):
            nc.gpsimd.load_library(MLP_LIB)
            # ---- constants / preloads ----
            cb = cpool.tile([P, 1], F32, tag="cb")
            nc.vector.memset(cb[:], -abs(float(alpha)))
            ident = cpool.tile([P, P], BF16, tag="ident")
            make_identity(nc, ident[:])
            iota2 = cpool.tile([P, P], BF16, tag="iota2")
            nc.gpsimd.iota(
                iota2[:],
                pattern=[[1, P]],
                base=0,
                channel_multiplier=0,
                allow_small_or_imprecise_dtypes=True,
            )
            ones_c = cpool.tile([P, 1], BF16, tag="ones_c")
            nc.vector.memset(ones_c[:], 1.0)
            wbs = cpool.tile([D + 1, D], BF16, tag="wbs")
            nc.sync.dma_start(out=wbs[:], in_=wb[:, :])
            zhT_sb = cpool.tile([D, cfg.npc], BF16, tag="zhT")
            nc.sync.dma_start(out=zhT_sb[:], in_=zhT[:, :])
            cidx_sb = cpool.tile([P, TU * 8], I16, tag="cidx")
            nc.sync.dma_start(out=cidx_sb[:], in_=cidx[:, :])
            rowt_sb = cpool.tile([P, TU], F32, tag="rowt")
            nc.sync.dma_start(out=rowt_sb[:], in_=rowt[:, :])

            # flatten chunks across superblocks for gather prefetch
            all_chunks = []
            for sb, chunks in enumerate(superblocks):
                for ch in chunks:
                    all_chunks.append((sb, ch))

            def issue_gather(ci):
                _, (g_, u0_, units_) = all_chunks[ci]
                nb_ = len(units_)
                gj_ = gpool.tile([P, CH, 128], BF16, tag="gj")
                nc.gpsimd.dma_gather(
                    gj_[:, :nb_, :],
                    tbl[:, g_, :],
                    cidx_sb[:, u0_ * 8 : (u0_ + nb_) * 8],
                    nb_ * P,
                    nb_ * P,
                    128,
                    elem_step=512,
                    queue_num=0,
                )
                return gj_

            def epilogue(sb, acc):
                accv = acc.rearrange("p (b c) -> p b c", c=D + 1)
                nsb = cfg.sb_blocks
                dpl = fpool.tile([P, nsb], F32, tag="dpl")
                nc.vector.tensor_scalar_add(
                    dpl[:, :], accv[:, :, D], EPS_DENOM
                )
                rr = fpool.tile([P, nsb], F32, tag="rr")
                nc.vector.reciprocal(out=rr[:, :], in_=dpl[:, :])
                m = fpool.tile([P, nsb, D + 1], BF16, tag="m")
                nc.vector.tensor_tensor(
                    out=m[:, :, 0:D],
                    in0=accv[:, :, 0:D],
                    in1=rr[:, :].to_broadcast([P, nsb, D]),
                    op=mybir.AluOpType.mult,
                )
                nc.vector.memset(m[:, :, D : D + 1], 1.0)
                o = fpool.tile([P, nsb, D], F32, tag="o")
                for j in range(nsb):
                    tp2 = ppool.tile([D + 1, P], BF16, tag="tp2", space="PSUM")
                    nc.tensor.transpose(
                        out=tp2[:], in_=m[:, j, :], identity=ident[:]
                    )
                    lhs = zpool.tile([D + 1, P], BF16, tag="lhs")
                    nc.vector.tensor_copy(out=lhs[:], in_=tp2[:])
                    y = ppool.tile([P, D], F32, tag="y", space="PSUM")
                    nc.tensor.matmul(
                        out=y[:], lhsT=lhs[:], rhs=wbs[:], start=True, stop=True
                    )
                    nc.scalar.copy(out=o[:, j, :], in_=y[:])
                nc.sync.dma_start(
                    out=out_v[:, sb * nsb : (sb + 1) * nsb, :], in_=o[:, :, :]
                )

            PREFETCH = 2
            gj_q = [
                issue_gather(i) for i in range(min(PREFETCH, len(all_chunks)))
            ]
            state = {"sb": -1, "acc": None}

            def consume(rec):
                # mask-multiply + message matmuls, one chunk behind the
                # produce stage so their exp-waits never head-of-line block
                # the next chunk's ready DVE/PE work
                sb, u0, units, gj, Es, Ss = rec
                if sb != state["sb"]:
                    if state["sb"] >= 0:
                        epilogue(state["sb"], state["acc"])
                    acc_t = apool.tile(
                        [P, cfg.sb_blocks * (D + 1)], F32, tag="acc"
                    )
                    state["acc"] = acc_t
                    state["sb"] = sb
                acc = state["acc"]
                nb = len(units)
                Ws = []
                for qi, i in enumerate(range(0, nb, 4)):
                    k = min(4, nb - i)
                    W = wpool.tile([P, 4, P], BF16, tag="W")
                    Ws.append(W)
                    nc.vector.tensor_tensor(
                        out=W[:, :k, :],
                        in0=Es[qi][:, :k, :],
                        in1=Ss[qi][:, :k, :],
                        op=mybir.AluOpType.mult,
                    )
                for u, (slot, first, last) in enumerate(units):
                    c0 = slot * (D + 1)
                    nc.tensor.matmul(
                        out=acc[:, c0 : c0 + D],
                        lhsT=Ws[u // 4][:, u % 4, :],
                        rhs=gj[:, u, D : 2 * D],
                        start=first,
                        stop=False,
                    )
                    # denominator: W row-sums via ones rhs; pads have
                    # rowt=200 so their W row is all zero
                    nc.tensor.matmul(
                        out=acc[:, c0 + D : c0 + D + 1],
                        lhsT=Ws[u // 4][:, u % 4, :],
                        rhs=ones_c[:],
                        start=False,
                        stop=last,
                    )

            pending = None
            for ci, (sb, (g, u0, units)) in enumerate(all_chunks):
                gj = gj_q.pop(0)
                if ci + PREFETCH < len(all_chunks):
                    gj_q.append(issue_gather(ci + PREFETCH))
                nb = len(units)
                # transpose batch: 7 bf16 [64,128] transposes fill one
                # 2KB psum bank; one copy moves them to SBUF for lhsT
                zcts = []
                for i in range(0, nb, 8):
                    k = min(8, nb - i)
                    tp8 = tpool.tile([D, 8, P], BF16, tag="tp", space="PSUM")
                    for j in range(k):
                        nc.tensor.transpose(
                            out=tp8[:, j, :],
                            in_=gj[:, i + j, 0:D],
                            identity=ident[:],
                        )
                    zct8 = zpool.tile([D, 8, P], BF16, tag="zct")
                    nc.vector.tensor_copy(out=zct8[:, :k, :], in_=tp8[:, :k, :])
                    zcts.append(zct8)
                # S masks first: no deps, keeps DVE queue flowing
                Ss = []
                for i in range(0, nb, 4):
                    k = min(4, nb - i)
                    S = epool.tile([P, 4, P], BF16, tag="S")
                    Ss.append(S)
                    for j in range(k):
                        u = i + j
                        # 4x DVE mode (scalar_tensor_tensor gets none)
                        nc.vector.tensor_scalar(
                            out=S[:, j, :],
                            in0=iota2[:],
                            scalar1=rowt_sb[:, u0 + u : u0 + u + 1],
                            scalar2=None,
                            op0=mybir.AluOpType.is_equal,
                        )
                Es = []
                for i in range(0, nb, 4):
                    k = min(4, nb - i)
                    mtp = mpool.tile([P, 512], F32, tag="mtp", space="PSUM")
                    for j in range(k):
                        u = i + j
                        slot, _, _ = units[u]
                        b = sb * cfg.sb_blocks + slot
                        nc.tensor.matmul(
                            out=mtp[:, j * P : (j + 1) * P],
                            lhsT=zcts[u // 8][:, u % 8, :],
                            rhs=zhT_sb[:, b * P : (b + 1) * P],
                            start=True,
                            stop=True,
                        )
                    E = epool.tile([P, 4, P], BF16, tag="E")
                    Es.append(E)
                    nc.scalar.activation(
                        out=E[:, :k, :],
                        in_=mtp[:, : k * P],
                        func=mybir.ActivationFunctionType.Exp,
                        bias=cb[:],
                        scale=float(alpha),
                    )
                rec = (sb, u0, units, gj, Es, Ss)
                if pending is not None:
                    consume(pending)
                pending = rec
            consume(pending)
            epilogue(state["sb"], state["acc"])

    nc.compile()
    return nc, TU


def _wrap16(a: np.ndarray) -> np.ndarray:
    w = a.reshape(-1, 16).T
    return np.ascontiguousarray(np.tile(w, (8, 1)))


def _to_bf16(a: np.ndarray) -> np.ndarray:
    import ml_dtypes

    return np.asarray(a, np.float32).astype(ml_dtypes.bfloat16)


def shard_inputs(cfg: Cfg, x, z, edge_index):
    D = cfg.d
    row = np.asarray(edge_index[0], dtype=np.int64)
    col = np.asarray(edge_index[1], dtype=np.int64)

    # degree-balanced block assignment: snake-deal nodes (by in-degree desc)
    # across all core*block bins, then repair-pass so nearly every
    # (bin, group) load is <= 4*128 edges (keeps units/segment at 4)
    nbins = cfg.n_cores * cfg.blocks
    total = nbins * P
    g_edge = (col & 3).astype(np.int64)
    deg4 = np.zeros((cfg.n_nodes, 4), np.int64)
    np.add.at(deg4, (row, g_edge), 1)
    deg = deg4.sum(axis=1)
    order = np.argsort(-deg, kind="stable")
    pos = np.arange(cfg.n_nodes)
    rnd = pos // nbins
    idx = pos % nbins
    bin_snake = np.where(rnd % 2 == 0, idx, nbins - 1 - idx)
    bin_of = np.empty(cfg.n_nodes, np.int64)
    bin_of[order] = bin_snake

    L = np.zeros((nbins, 4), np.int64)
    for gg in range(4):
        L[:, gg] = np.bincount(
            bin_of, weights=deg4[:, gg].astype(np.float64), minlength=nbins
        ).astype(np.int64)
    # swap-repair toward per-(bin,g) caps: most bins capped at 4 units worth
    # of edges; a few designated "tall" bins (cap 5 units) concentrate the
    # overflow so nearly every segment ends at U=4.
    caps = np.full((nbins, 4), 4 * P, np.int64)
    if nbins >= 64:
        taken = np.zeros(nbins, bool)
        for gg in range(4):
            cnt = 0
            for bb in np.argsort(-L[:, gg]):
                if not taken[bb]:
                    taken[bb] = True
                    caps[bb, gg] = 5 * P
                    cnt += 1
                    if cnt == 24:
                        break
    bin_rows = [list(np.where(bin_of == bb)[0]) for bb in range(nbins)]
    for _ in range(40):
        over = np.where((L > caps).any(axis=1))[0]
        if over.size == 0:
            break
        swapped = 0
        for bb in over:
            guard = 0
            while (L[bb] > caps[bb]).any() and guard < 30:
                guard += 1
                gbad = int(np.argmax(L[bb] - caps[bb]))
                r1 = max(bin_rows[bb], key=lambda r: deg4[r, gbad])
                d1 = deg4[r1]
                done = False
                slack = caps[:, gbad] - L[:, gbad]
                for dst in np.argsort(-slack)[:100]:
                    dst = int(dst)
                    if dst == bb or slack[dst] <= 0:
                        continue
                    cand2 = sorted(
                        bin_rows[dst], key=lambda r: deg4[r, gbad]
                    )[:6]
                    for r2 in cand2:
                        d2 = deg4[r2]
                        newL_dst = L[dst] + d1 - d2
                        if (newL_dst <= caps[dst]).all() and d1[gbad] > d2[gbad]:
                            bin_rows[bb].remove(r1)
                            bin_rows[dst].remove(r2)
                            bin_rows[bb].append(r2)
                            bin_rows[dst].append(r1)
                            L[bb] += d2 - d1
                            L[dst] = newL_dst
                            swapped += 1
                            done = True
                            break
                    if done:
                        break
                if not done:
                    break
        if swapped == 0:
            break

    # pair bins into (core, block) slots by unit-pattern so the per-(b,g)
    # max over cores matches each bin's own ceil pattern
    ceil_pat = (L + P - 1) // P  # [nbins, 4]
    pat_key = (
        ceil_pat[:, 0] * 1000000
        + ceil_pat[:, 1] * 10000
        + ceil_pat[:, 2] * 100
        + ceil_pat[:, 3]
    )
    bin_order = np.argsort(pat_key, kind="stable")
    # slot s (0..97) of core k gets bin bin_order[s*8+k]
    new_pos = np.empty(cfg.n_nodes, np.int64)
    for s in range(cfg.blocks):
        for k in range(cfg.n_cores):
            bb = int(bin_order[s * cfg.n_cores + k])
            rs = bin_rows[bb]
            base = (k * cfg.blocks + s) * P
            new_pos[rs] = base + np.arange(len(rs))
    perm = new_pos
    inv_perm = np.full(total, -1, np.int64)
    inv_perm[new_pos] = np.arange(cfg.n_nodes)

    row = perm[row]
    core = row // cfg.npc
    local = row - core * cfg.npc
    blk = local >> 7
    rt = (local & 127).astype(np.float32)
    g = (col & 3).astype(np.int64)
    lidx = (col >> 2).astype(np.int64)

    nbg = cfg.blocks * cfg.n_groups
    key = (blk * cfg.n_groups + g).astype(np.int64)
    counts = np.zeros((cfg.n_cores, nbg), np.int64)
    for k in range(cfg.n_cores):
        sel = core == k
        counts[k] = np.bincount(key[sel], minlength=nbg)
    cmax = counts.max(axis=0).reshape(cfg.blocks, cfg.n_groups)
    U = (cmax + P - 1) // P
    # every superblock needs at least one unit so its psum acc region is
    # started (start=True zeroes the whole bank)
    n_sb = cfg.blocks // cfg.sb_blocks
    for sb in range(n_sb):
        sl = slice(sb * cfg.sb_blocks, (sb + 1) * cfg.sb_blocks)
        if U[sl].sum() == 0:
            U[sb * cfg.sb_blocks, 0] = 1
    TU = int(U.sum())

    # global unit offsets in stream order (sb -> g -> b)
    seg_u0 = np.zeros((cfg.blocks, cfg.n_groups), np.int64)
    u0 = 0
    n_sb = cfg.blocks // cfg.sb_blocks
    for sb in range(n_sb):
        for gg in range(cfg.n_groups):
            for b in range(sb * cfg.sb_blocks, (sb + 1) * cfg.sb_blocks):
                seg_u0[b, gg] = u0
                u0 += int(U[b, gg])
    assert u0 == TU
    T = TU * P

    cidx_cores, rowt_cores = [], []
    for k in range(cfg.n_cores):
        sel = core == k
        kk = key[sel]
        order = np.argsort(kk, kind="stable")
        ks = kk[order]
        rank = np.arange(ks.size) - np.searchsorted(ks, ks)
        b_s = ks // cfg.n_groups
        g_s = ks % cfg.n_groups
        tokpos = seg_u0[b_s, g_s] * P + rank
        ci = np.full(T, cfg.null_lidx, np.int16)
        rw = np.full(T, 200.0, np.float32)  # pad rowt: never matches iota
        ci[tokpos] = lidx[sel][order].astype(np.int16)
        rw[tokpos] = rt[sel][order]
        cidx_cores.append(_wrap16(ci))
        rowt_cores.append(np.ascontiguousarray(rw.reshape(-1, P).T))

    zf = np.asarray(z, np.float32)
    nrm = np.maximum(np.sqrt((zf * zf).sum(axis=1)), 1e-9)
    zh = zf / nrm[:, None]
    xf = np.asarray(x, np.float32)

    import ml_dtypes

    tbl = np.zeros((cfg.table_rows, 128), ml_dtypes.bfloat16)
    tbl[: cfg.n_nodes, 0:D] = _to_bf16(zh)
    tbl[: cfg.n_nodes, D : 2 * D] = _to_bf16(xf)
    tbl = tbl.reshape(cfg.group_cap, 4, 128)

    # zh rows in permuted order (phantom tail rows stay zero)
    zh_pad = np.zeros((total, D), np.float32)
    real = inv_perm >= 0
    zh_pad[real] = zh[inv_perm[real]]

    in_maps = []
    for k in range(cfg.n_cores):
        zslab = zh_pad[k * cfg.npc : (k + 1) * cfg.npc]
        in_maps.append(
            {
                "tbl": tbl,
                "zhT": _to_bf16(np.ascontiguousarray(zslab.T)),
                "cidx": cidx_cores[k],
                "rowt": rowt_cores[k],
            }
        )
    return in_maps, U, TU, perm


def prep_wb(W, b):
    wb = np.concatenate(
        [np.asarray(W, np.float32).T, np.asarray(b, np.float32)[None, :]], axis=0
    )
    return _to_bf16(np.ascontiguousarray(wb))


def run(cfg: Cfg, x, edge_index, z, W, b, alpha, bias_edge, trace=False):
    from concourse.bass_utils import run_bass_kernel_spmd

    in_maps, U, TU, perm = shard_inputs(cfg, x, z, edge_index)
    wbv = prep_wb(W, b)
    for m in in_maps:
        m["wb"] = wbv
    nc, _ = build_program(cfg, U, float(np.asarray(alpha)))
    core_ids = list(range(cfg.n_cores))
    res = run_bass_kernel_spmd(nc, in_maps, core_ids, trace=trace)
    outs = [res.results[k]["out"] for k in core_ids]
    out = np.concatenate(outs, axis=0)[perm]
    return np.ascontiguousarray(out).astype(np.float32), res


def kernel(**inputs) -> np.ndarray:
    out, _ = run(
        FULL,
        inputs["x"],
        inputs["edge_index"],
        inputs["z"],
        inputs["W"],
        inputs["b"],
        inputs["alpha"],
        inputs["bias_edge"],
    )
    return out


# revision 10
# speedup vs baseline: 1.1568x; 1.1429x over previous
"""Trainium2 Bass kernel for nn_DiracGraphConv (GNN edge-softmax message passing).

Block-matmul design (v2, no scatter):
  - Host normalizes z (zh), shards edges by destination: core k owns rows
    [k*12544, (k+1)*12544); blocks of 128 rows; superblocks of 7 blocks.
  - Tokens (edges) are grouped by (block b, col-group g=col&3) and padded to
    128-token units U; the per-(b,g) unit count is the max over cores so all 8
    cores share one SPMD program.
  - Per chunk (<=CH_U units, single g): one dma_gather of 512B rows
    [zh(64) | x(64) | 1 | pad] bf16 from the node table (window = cols==g mod 4
    via elem_step).
  - Per unit: PE-transpose the gathered zh -> zh_c^T, PE matmul with the
    block's host-transposed zh slab -> Mt[tok,row] (all-pairs logits) in PSUM;
    batched exp on ACT; one-hot row mask S built by iota/is_equal on DVE;
    W = exp(Mt)*S; PE matmul W^T @ [x|1] accumulates messages+denom for the
    block directly in PSUM. Pad tokens gather an all-zero table row so they
    contribute nothing.
  - Superblock epilogue: out = (msg/(denom+eps)) @ W^T + b via PE.
"""

import sys

sys.path.insert(0, "/opt/trn_rl_repo")

from dataclasses import dataclass

import numpy as np

from concourse import bacc, bass, mybir, tile
from concourse.library_config import mlp as MLP_LIB
from concourse.masks import make_identity

P = 128
F32 = mybir.dt.float32
BF16 = mybir.dt.bfloat16
I16 = mybir.dt.int16
EPS_DENOM = 1e-9


@dataclass(frozen=True)
class Cfg:
    n_cores: int = 8
    n_nodes: int = 100000
    d: int = 64
    npc: int = 12544  # nodes per core; cores 0-6 full, core 7 partial
    n_groups: int = 4
    sb_blocks: int = 7  # blocks per superblock (acc psum bank holds 7*65 f32)
    # max 128-token units per gather chunk: HW SWDGE handles <1024
    # descriptors per instruction, so 7*128=896 is the clean maximum
    ch_u: int = 7
    ring: int = 65536  # SWDGE descriptor ring bytes (4096 descs)
    group_cap: int = 25088  # local col index cap; null idx = group_cap-1

    @property
    def blocks(self) -> int:
        return self.npc // P

    @property
    def null_lidx(self) -> int:
        # null table row = 4*null_lidx + g >= n_nodes must hold
        assert 4 * (self.group_cap - 1) >= self.n_nodes
        return self.group_cap - 1

    @property
    def table_rows(self) -> int:
        return 4 * self.group_cap


FULL = Cfg()


def make_structure(U: np.ndarray, cfg: Cfg):
    """U[b, g] = number of 128-token units for (block, group).
    Returns (TU, superblocks) where superblocks is a list (one per sb) of
    chunk lists; each chunk is (g, u0, units) with units a list of
    (block_local_slot, start_flag, stop_flag) per unit."""
    nb_blocks = cfg.blocks
    n_sb = nb_blocks // cfg.sb_blocks
    # global unit index in stream order: sb -> g -> b -> u
    TU = int(U.sum())
    # start/stop flags are per-superblock: the first matmul into an acc bank
    # zeroes the whole 2KB region, the last one closes the group
    superblocks = []
    u0 = 0
    for sb in range(n_sb):
        blocks = range(sb * cfg.sb_blocks, (sb + 1) * cfg.sb_blocks)
        sb_total = int(U[sb * cfg.sb_blocks : (sb + 1) * cfg.sb_blocks].sum())
        assert sb_total > 0, "superblock with no units"
        chunks = []
        seen = 0
        for g in range(cfg.n_groups):
            units = []  # (slot, is_first, is_last) in stream order
            for b in blocks:
                for _ in range(int(U[b, g])):
                    units.append(
                        (b - sb * cfg.sb_blocks, seen == 0, seen == sb_total - 1)
                    )
                    seen += 1
            # split into chunks of <= ch_u
            i = 0
            while i < len(units):
                k = min(cfg.ch_u, len(units) - i)
                chunks.append((g, u0 + i, units[i : i + k]))
                i += k
            u0 += len(units)
        assert seen == sb_total
        superblocks.append(chunks)
    return TU, superblocks


def build_program(cfg: Cfg, U: np.ndarray, alpha: float):
    TU, superblocks = make_structure(U, cfg)
    D = cfg.d
    CH = cfg.ch_u

    nc = bacc.Bacc(
        "TRN2",
        target_bir_lowering=False,
        debug=False,
        num_swdge_queues=2,
        dynamic_dma_scratch_size=cfg.ring,
    )

    tbl = nc.dram_tensor(
        "tbl", [cfg.group_cap, 4, 128], BF16, kind="ExternalInput"
    ).ap()
    zhT = nc.dram_tensor("zhT", [D, cfg.npc], BF16, kind="ExternalInput").ap()
    cidx = nc.dram_tensor("cidx", [P, TU * 8], I16, kind="ExternalInput").ap()
    rowt = nc.dram_tensor("rowt", [P, TU], F32, kind="ExternalInput").ap()
    wb = nc.dram_tensor("wb", [D + 1, D], BF16, kind="ExternalInput").ap()
    out = nc.dram_tensor("out", [cfg.npc, D], F32, kind="ExternalOutput").ap()
    out_v = out.rearrange("(b p) d -> p b d", p=P)

    with tile.TileContext(nc) as tc:
        with (
            tc.tile_pool(name="const", bufs=1) as cpool,
            tc.tile_pool(name="gath", bufs=5) as gpool,
            tc.tile_pool(name="esw", bufs=6) as epool,
            tc.tile_pool(name="wp", bufs=4) as wpool,
            tc.tile_pool(name="zct", bufs=3) as zpool,
            tc.tile_pool(name="fin", bufs=2) as fpool,
            tc.tile_pool(name="mt", bufs=2, space="PSUM") as mpool,
            tc.tile_pool(name="tp", bufs=2, space="PSUM") as tpool,
            tc.tile_pool(name="acc", bufs=2, space="PSUM") as apool,
            tc.tile_pool(name="ep", bufs=1, space="PSUM") as ppool,
        ):
            nc.gpsimd.load_library(MLP_LIB)
            # ---- constants / preloads ----
            cb = cpool.tile([P, 1], F32, tag="cb")
            nc.vector.memset(cb[:], -abs(float(alpha)))
            ident = cpool.tile([P, P], BF16, tag="ident")
            make_identity(nc, ident[:])
            iota2 = cpool.tile([P, P], BF16, tag="iota2")
            nc.gpsimd.iota(
                iota2[:],
                pattern=[[1, P]],
                base=0,
                channel_multiplier=0,
                allow_small_or_imprecise_dtypes=True,
            )
            ones_c = cpool.tile([P, 1], BF16, tag="ones_c")
            nc.vector.memset(ones_c[:], 1.0)
            wbs = cpool.tile([D + 1, D], BF16, tag="wbs")
            nc.sync.dma_start(out=wbs[:], in_=wb[:, :])
            zhT_sb = cpool.tile([D, cfg.npc], BF16, tag="zhT")
            nc.sync.dma_start(out=zhT_sb[:], in_=zhT[:, :])
            cidx_sb = cpool.tile([P, TU * 8], I16, tag="cidx")
            nc.sync.dma_start(out=cidx_sb[:], in_=cidx[:, :])
            rowt_sb = cpool.tile([P, TU], F32, tag="rowt")
            nc.sync.dma_start(out=rowt_sb[:], in_=rowt[:, :])

            # flatten chunks across superblocks for gather prefetch
            all_chunks = []
            for sb, chunks in enumerate(superblocks):
                for ch in chunks:
                    all_chunks.append((sb, ch))

            def issue_gather(ci):
                _, (g_, u0_, units_) = all_chunks[ci]
                nb_ = len(units_)
                gj_ = gpool.tile([P, CH, 128], BF16, tag="gj")
                nc.gpsimd.dma_gather(
                    gj_[:, :nb_, :],
                    tbl[:, g_, :],
                    cidx_sb[:, u0_ * 8 : (u0_ + nb_) * 8],
                    nb_ * P,
                    nb_ * P,
                    128,
                    elem_step=512,
                    queue_num=ci % 2,
                )
                return gj_

            def epilogue(sb, acc):
                accv = acc.rearrange("p (b c) -> p b c", c=D + 1)
                nsb = cfg.sb_blocks
                dpl = fpool.tile([P, nsb], F32, tag="dpl")
                nc.vector.tensor_scalar_add(
                    dpl[:, :], accv[:, :, D], EPS_DENOM
                )
                rr = fpool.tile([P, nsb], F32, tag="rr")
                nc.vector.reciprocal(out=rr[:, :], in_=dpl[:, :])
                m = fpool.tile([P, nsb, D + 1], BF16, tag="m")
                nc.vector.tensor_tensor(
                    out=m[:, :, 0:D],
                    in0=accv[:, :, 0:D],
                    in1=rr[:, :].to_broadcast([P, nsb, D]),
                    op=mybir.AluOpType.mult,
                )
                nc.vector.memset(m[:, :, D : D + 1], 1.0)
                o = fpool.tile([P, nsb, D], F32, tag="o")
                for j in range(nsb):
                    tp2 = ppool.tile([D + 1, P], BF16, tag="tp2", space="PSUM")
                    nc.tensor.transpose(
                        out=tp2[:], in_=m[:, j, :], identity=ident[:]
                    )
                    lhs = zpool.tile([D + 1, P], BF16, tag="lhs")
                    nc.vector.tensor_copy(out=lhs[:], in_=tp2[:])
                    y = ppool.tile([P, D], F32, tag="y", space="PSUM")
                    nc.tensor.matmul(
                        out=y[:], lhsT=lhs[:], rhs=wbs[:], start=True, stop=True
                    )
                    nc.scalar.copy(out=o[:, j, :], in_=y[:])
                nc.sync.dma_start(
                    out=out_v[:, sb * nsb : (sb + 1) * nsb, :], in_=o[:, :, :]
                )

            PREFETCH = 2
            gj_q = [
                issue_gather(i) for i in range(min(PREFETCH, len(all_chunks)))
            ]
            state = {"sb": -1, "acc": None}

            def consume(rec):
                # mask-multiply + message matmuls, one chunk behind the
                # produce stage so their exp-waits never head-of-line block
                # the next chunk's ready DVE/PE work
                sb, u0, units, gj, Es, Ss = rec
                if sb != state["sb"]:
                    if state["sb"] >= 0:
                        epilogue(state["sb"], state["acc"])
                    acc_t = apool.tile(
                        [P, cfg.sb_blocks * (D + 1)], F32, tag="acc"
                    )
                    state["acc"] = acc_t
                    state["sb"] = sb
                acc = state["acc"]
                nb = len(units)
                Ws = []
                for qi, i in enumerate(range(0, nb, 4)):
                    k = min(4, nb - i)
                    W = wpool.tile([P, 4, P], BF16, tag="W")
                    Ws.append(W)
                    nc.vector.tensor_tensor(
                        out=W[:, :k, :],
                        in0=Es[qi][:, :k, :],
                        in1=Ss[qi][:, :k, :],
                        op=mybir.AluOpType.mult,
                    )
                for u, (slot, first, last) in enumerate(units):
                    c0 = slot * (D + 1)
                    nc.tensor.matmul(
                        out=acc[:, c0 : c0 + D],
                        lhsT=Ws[u // 4][:, u % 4, :],
                        rhs=gj[:, u, D : 2 * D],
                        start=first,
                        stop=False,
                    )
                    # denominator: W row-sums via ones rhs; pads have
                    # rowt=200 so their W row is all zero
                    nc.tensor.matmul(
                        out=acc[:, c0 + D : c0 + D + 1],
                        lhsT=Ws[u // 4][:, u % 4, :],
                        rhs=ones_c[:],
                        start=False,
                        stop=last,
                    )

            pending = None
            for ci, (sb, (g, u0, units)) in enumerate(all_chunks):
                gj = gj_q.pop(0)
                if ci + PREFETCH < len(all_chunks):
                    gj_q.append(issue_gather(ci + PREFETCH))
                nb = len(units)
                # transpose batch: 7 bf16 [64,128] transposes fill one
                # 2KB psum bank; one copy moves them to SBUF for lhsT
                zcts = []
                for i in range(0, nb, 8):
                    k = min(8, nb - i)
                    tp8 = tpool.tile([D, 8, P], BF16, tag="tp", space="PSUM")
                    for j in range(k):
                        nc.tensor.transpose(
                            out=tp8[:, j, :],
                            in_=gj[:, i + j, 0:D],
                            identity=ident[:],
                        )
                    zct8 = zpool.tile([D, 8, P], BF16, tag="zct")
                    nc.vector.tensor_copy(out=zct8[:, :k, :], in_=tp8[:, :k, :])
                    zcts.append(zct8)
                # S masks first: no deps, keeps DVE queue flowing
                Ss = []
                for i in range(0, nb, 4):
                    k = min(4, nb - i)
                    S = epool.tile([P, 4, P], BF16, tag="S")
                    Ss.append(S)
                    for j in range(k):
                        u = i + j
                        # 4x DVE mode (scalar_tensor_tensor gets none)
                        nc.vector.tensor_scalar(
                            out=S[:, j, :],
                            in0=iota2[:],
                            scalar1=rowt_sb[:, u0 + u : u0 + u + 1],
                            scalar2=None,
                            op0=mybir.AluOpType.is_equal,
                        )
                Es = []
                for i in range(0, nb, 4):
                    k = min(4, nb - i)
                    mtp = mpool.tile([P, 512], F32, tag="mtp", space="PSUM")
                    for j in range(k):
                        u = i + j
                        slot, _, _ = units[u]
                        b = sb * cfg.sb_blocks + slot
                        nc.tensor.matmul(
                            out=mtp[:, j * P : (j + 1) * P],
                            lhsT=zcts[u // 8][:, u % 8, :],
                            rhs=zhT_sb[:, b * P : (b + 1) * P],
                            start=True,
                            stop=True,
                        )
                    E = epool.tile([P, 4, P], BF16, tag="E")
                    Es.append(E)
                    nc.scalar.activation(
                        out=E[:, :k, :],
                        in_=mtp[:, : k * P],
                        func=mybir.ActivationFunctionType.Exp,
                        bias=cb[:],
                        scale=float(alpha),
                    )
                rec = (sb, u0, units, gj, Es, Ss)
                if pending is not None:
                    consume(pending)
                pending = rec
            consume(pending)
            epilogue(state["sb"], state["acc"])

    nc.compile()
    return nc, TU


def _wrap16(a: np.ndarray) -> np.ndarray:
    w = a.reshape(-1, 16).T
    return np.ascontiguousarray(np.tile(w, (8, 1)))


def _to_bf16(a: np.ndarray) -> np.ndarray:
    import ml_dtypes

    return np.asarray(a, np.float32).astype(ml_dtypes.bfloat16)


def shard_inputs(cfg: Cfg, x, z, edge_index):
    D = cfg.d
    row = np.asarray(edge_index[0], dtype=np.int64)
    col = np.asarray(edge_index[1], dtype=np.int64)

    # degree-balanced block assignment: snake-deal nodes (by in-degree desc)
    # across all core*block bins, then repair-pass so nearly every
    # (bin, group) load is <= 4*128 edges (keeps units/segment at 4)
    nbins = cfg.n_cores * cfg.blocks
    total = nbins * P
    g_edge = (col & 3).astype(np.int64)
    deg4 = np.zeros((cfg.n_nodes, 4), np.int64)
    np.add.at(deg4, (row, g_edge), 1)
    deg = deg4.sum(axis=1)
    order = np.argsort(-deg, kind="stable")
    pos = np.arange(cfg.n_nodes)
    rnd = pos // nbins
    idx = pos % nbins
    bin_snake = np.where(rnd % 2 == 0, idx, nbins - 1 - idx)
    bin_of = np.empty(cfg.n_nodes, np.int64)
    bin_of[order] = bin_snake

    L = np.zeros((nbins, 4), np.int64)
    for gg in range(4):
        L[:, gg] = np.bincount(
            bin_of, weights=deg4[:, gg].astype(np.float64), minlength=nbins
        ).astype(np.int64)
    # swap-repair toward per-(bin,g) caps: most bins capped at 4 units worth
    # of edges; a few designated "tall" bins (cap 5 units) concentrate the
    # overflow so nearly every segment ends at U=4.
    caps = np.full((nbins, 4), 4 * P, np.int64)
    if nbins >= 64:
        taken = np.zeros(nbins, bool)
        for gg in range(4):
            cnt = 0
            for bb in np.argsort(-L[:, gg]):
                if not taken[bb]:
                    taken[bb] = True
                    caps[bb, gg] = 5 * P
                    cnt += 1
                    if cnt == 24:
                        break
    bin_rows = [list(np.where(bin_of == bb)[0]) for bb in range(nbins)]
    for _ in range(40):
        over = np.where((L > caps).any(axis=1))[0]
        if over.size == 0:
            break
        swapped = 0
        for bb in over:
            guard = 0
            while (L[bb] > caps[bb]).any() and guard < 30:
                guard += 1
                gbad = int(np.argmax(L[bb] - caps[bb]))
                r1 = max(bin_rows[bb], key=lambda r: deg4[r, gbad])
                d1 = deg4[r1]
                done = False
                slack = caps[:, gbad] - L[:, gbad]
                for dst in np.argsort(-slack)[:100]:
                    dst = int(dst)
                    if dst == bb or slack[dst] <= 0:
                        continue
                    cand2 = sorted(
                        bin_rows[dst], key=lambda r: deg4[r, gbad]
                    )[:6]
                    for r2 in cand2:
                        d2 = deg4[r2]
                        newL_dst = L[dst] + d1 - d2
                        if (newL_dst <= caps[dst]).all() and d1[gbad] > d2[gbad]:
                            bin_rows[bb].remove(r1)
                            bin_rows[dst].remove(r2)
                            bin_rows[bb].append(r2)
                            bin_rows[dst].append(r1)
                            L[bb] += d2 - d1
                            L[dst] = newL_dst
                            swapped += 1
                            done = True
                            break
                    if done:
                        break
                if not done:
                    break
        if swapped == 0:
            break

    # pair bins into (core, block) slots by unit-pattern so the per-(b,g)
    # max over cores matches each bin's own ceil pattern
    ceil_pat = (L + P - 1) // P  # [nbins, 4]
    pat_key = (
        ceil_pat[:, 0] * 1000000
        + ceil_pat[:, 1] * 10000
        + ceil_pat[:, 2] * 100
        + ceil_pat[:, 3]
    )
    bin_order = np.argsort(pat_key, kind="stable")
    # slot s (0..97) of core k gets bin bin_order[s*8+k]
    new_pos = np.empty(cfg.n_nodes, np.int64)
    for s in range(cfg.blocks):
        for k in range(cfg.n_cores):
            bb = int(bin_order[s * cfg.n_cores + k])
            rs = bin_rows[bb]
            base = (k * cfg.blocks + s) * P
            new_pos[rs] = base + np.arange(len(rs))
    perm = new_pos
    inv_perm = np.full(total, -1, np.int64)
    inv_perm[new_pos] = np.arange(cfg.n_nodes)

    row = perm[row]
    core = row // cfg.npc
    local = row - core * cfg.npc
    blk = local >> 7
    rt = (local & 127).astype(np.float32)
    g = (col & 3).astype(np.int64)
    lidx = (col >> 2).astype(np.int64)

    nbg = cfg.blocks * cfg.n_groups
    key = (blk * cfg.n_groups + g).astype(np.int64)
    counts = np.zeros((cfg.n_cores, nbg), np.int64)
    for k in range(cfg.n_cores):
        sel = core == k
        counts[k] = np.bincount(key[sel], minlength=nbg)
    cmax = counts.max(axis=0).reshape(cfg.blocks, cfg.n_groups)
    U = (cmax + P - 1) // P
    # every superblock needs at least one unit so its psum acc region is
    # started (start=True zeroes the whole bank)
    n_sb = cfg.blocks // cfg.sb_blocks
    for sb in range(n_sb):
        sl = slice(sb * cfg.sb_blocks, (sb + 1) * cfg.sb_blocks)
        if U[sl].sum() == 0:
            U[sb * cfg.sb_blocks, 0] = 1
    TU = int(U.sum())

    # global unit offsets in stream order (sb -> g -> b)
    seg_u0 = np.zeros((cfg.blocks, cfg.n_groups), np.int64)
    u0 = 0
    n_sb = cfg.blocks // cfg.sb_blocks
    for sb in range(n_sb):
        for gg in range(cfg.n_groups):
            for b in range(sb * cfg.sb_blocks, (sb + 1) * cfg.sb_blocks):
                seg_u0[b, gg] = u0
                u0 += int(U[b, gg])
    assert u0 == TU
    T = TU * P

    cidx_cores, rowt_cores = [], []
    for k in range(cfg.n_cores):
        sel = core == k
        kk = key[sel]
        order = np.argsort(kk, kind="stable")
        ks = kk[order]
        rank = np.arange(ks.size) - np.searchsorted(ks, ks)
        b_s = ks // cfg.n_groups
        g_s = ks % cfg.n_groups
        tokpos = seg_u0[b_s, g_s] * P + rank
        ci = np.full(T, cfg.null_lidx, np.int16)
        rw = np.full(T, 200.0, np.float32)  # pad rowt: never matches iota
        ci[tokpos] = lidx[sel][order].astype(np.int16)
        rw[tokpos] = rt[sel][order]
        cidx_cores.append(_wrap16(ci))
        rowt_cores.append(np.ascontiguousarray(rw.reshape(-1, P).T))

    zf = np.asarray(z, np.float32)
    nrm = np.maximum(np.sqrt((zf * zf).sum(axis=1)), 1e-9)
    zh = zf / nrm[:, None]
    xf = np.asarray(x, np.float32)

    import ml_dtypes

    tbl = np.zeros((cfg.table_rows, 128), ml_dtypes.bfloat16)
    tbl[: cfg.n_nodes, 0:D] = _to_bf16(zh)
    tbl[: cfg.n_nodes, D : 2 * D] = _to_bf16(xf)
    tbl = tbl.reshape(cfg.group_cap, 4, 128)

    # zh rows in permuted order (phantom tail rows stay zero)
    zh_pad = np.zeros((total, D), np.float32)
    real = inv_perm >= 0
    zh_pad[real] = zh[inv_perm[real]]

    in_maps = []
    for k in range(cfg.n_cores):
        zslab = zh_pad[k * cfg.npc : (k + 1) * cfg.npc]
        in_maps.append(
            {
                "tbl": tbl,
                "zhT": _to_bf16(np.ascontiguousarray(zslab.T)),
                "cidx": cidx_cores[k],
                "rowt": rowt_cores[k],
            }
        )
    return in_maps, U, TU, perm


def prep_wb(W, b):
    wb = np.concatenate(
        [np.asarray(W, np.float32).T, np.asarray(b, np.float32)[None, :]], axis=0
    )
    return _to_bf16(np.ascontiguousarray(wb))


def run(cfg: Cfg, x, edge_index, z, W, b, alpha, bias_edge, trace=False):
    from concourse.bass_utils import run_bass_kernel_spmd

    in_maps, U, TU, perm = shard_inputs(cfg, x, z, edge_index)
    wbv = prep_wb(W, b)
    for m in in_maps:
        m["wb"] = wbv
    nc, _ = build_program(cfg, U, float(np.asarray(alpha)))
    core_ids = list(range(cfg.n_cores))
    res = run_bass_kernel_spmd(nc, in_maps, core_ids, trace=trace)
    outs = [res.results[k]["out"] for k in core_ids]
    out = np.concatenate(outs, axis=0)[perm]
    return np.ascontiguousarray(out).astype(np.float32), res


def kernel(**inputs) -> np.ndarray:
    out, _ = run(
        FULL,
        inputs["x"],
        inputs["edge_index"],
        inputs["z"],
        inputs["W"],
        inputs["b"],
        inputs["alpha"],
        inputs["bias_edge"],
    )
    return out
